# revision 2
# baseline (speedup 1.0000x reference)
"""Bahdanau additive attention on 8 Trainium2 NeuronCores (Bass/Tile).

Reference computation (per batch b):
    wq   = query @ wa_w.T + wa_b                      # [1, H]
    uk   = keys  @ ua_w.T + ua_b                      # [L, H]
    s    = tanh(wq + uk) @ va_w.T + va_b              # [L]
    s    = where(mask, -inf, s)
    w    = softmax(s)                                 # [L]
    ctx  = w @ keys                                   # [1, H]

Sharding: data-parallel over batch B=32 -> 4 batches per core; small
weights replicated.  The heavy matmuls run on the PE in float32r
(full-rate fp32, tf32-like operand rounding).

Device-side structure (per core, BC=4 batches):
  - The big matmul computes uk^T [k, l] so that the per-batch
    wq[k]+wa_b[k]+ua_b[k] (tiny; precomputed on host) becomes a
    per-partition ACT bias fused into the tanh.
  - scores = va . tanh(.) is a PE matmul with va as a [128,1] stationary;
    the mask bias is added during the PSUM->SBUF scores copy.
  - softmax runs on partition 0 ([1, 2048] per batch); exp and its sum are
    fused via ACT accum_out; normalization is folded into the final context
    copy (softmax is scale-invariant, va_b dropped for the same reason).
  - pass 2 (ctx = e @ keys) uses natural-layout keys as moving operand with
    the unnormalized exp weights (PE-transposed to the partition dim) as
    stationary.
"""

import os
import numpy as np
from contextlib import ExitStack

import concourse.bass as bass  # noqa: F401
import concourse.bacc as bacc
import concourse.tile as tile
from concourse import mybir
from concourse.bass_utils import run_bass_kernel_spmd

B, L, H = 32, 2048, 1024
NCORES = 8
BC = B // NCORES          # batches per core
HC = H // 128             # 128-chunks of the hidden dim
LT = 512                  # l-tile width for pass 1
NLT = L // LT
NLC = L // 128            # 128-chunks of l for pass 2

F32 = mybir.dt.float32
F32R = mybir.dt.float32r
AF = mybir.ActivationFunctionType
AX = mybir.AxisListType
OP = mybir.AluOpType

_nc = None
LAST_RESULT = None


def _body(nc, tc, ctx, d):
    consts = ctx.enter_context(tc.tile_pool(name="consts", bufs=1))
    kpool = ctx.enter_context(tc.tile_pool(name="kT", bufs=3))
    tpool = ctx.enter_context(tc.tile_pool(name="tk", bufs=3))
    knpool = ctx.enter_context(tc.tile_pool(name="kN", bufs=4))
    small = ctx.enter_context(tc.tile_pool(name="small", bufs=2))
    p_uk = ctx.enter_context(tc.tile_pool(name="p_uk", bufs=2, space="PSUM"))
    p_sc = ctx.enter_context(tc.tile_pool(name="p_sc", bufs=2, space="PSUM"))
    p_wt = ctx.enter_context(tc.tile_pool(name="p_wt", bufs=2, space="PSUM"))
    p_ctx = ctx.enter_context(tc.tile_pool(name="p_ctx", bufs=2, space="PSUM"))

    # ---- constants / weights ----
    uawT = consts.tile([128, HC * H], F32R)
    nc.sync.dma_start(uawT[:], d["uawT"].bitcast(F32R))
    vaT = consts.tile([128, HC], F32R)
    nc.sync.dma_start(vaT[:], d["vaT"].bitcast(F32R))
    one = consts.tile([1, 1], F32)
    nc.sync.dma_start(one[:], d["one"])
    biasT = consts.tile([128, HC * BC], F32)
    nc.sync.dma_start(biasT[:], d["biasT"])

    # ---- main per-batch pipeline ----
    for b in range(BC):
        scores = small.tile([1, L], F32, tag="scores")
        mb = small.tile([1, L], F32, tag="mb")
        nc.sync.dma_start(mb[:], d["maskb"][b : b + 1, :])

        # pass 1: uk^T tiles -> tanh -> scores
        for lt in range(NLT):
            kT = kpool.tile([128, HC, LT], F32R)
            nc.sync.dma_start(
                kT[:],
                d["keysT"][b, :, lt * LT : (lt + 1) * LT]
                .rearrange("(hc p) l -> p hc l", p=128)
                .bitcast(F32R),
            )
            ps = p_sc.tile([1, LT], F32)
            for kc in range(HC):
                pu = p_uk.tile([128, LT], F32)
                for hc in range(HC):
                    nc.tensor.matmul(
                        pu[:],
                        uawT[:, hc * H + kc * 128 : hc * H + (kc + 1) * 128],
                        kT[:, hc, :],
                        start=(hc == 0),
                        stop=(hc == HC - 1),
                    )
                tk = tpool.tile([128, LT], F32R)
                nc.scalar.activation(
                    tk[:], pu[:], AF.Tanh,
                    bias=biasT[:, kc * BC + b : kc * BC + b + 1], scale=1.0,
                )
                nc.tensor.matmul(
                    ps[:], vaT[:, kc : kc + 1], tk[:],
                    start=(kc == 0), stop=(kc == HC - 1),
                )
            # scores = ps + mask_bias
            nc.vector.tensor_add(
                scores[0:1, lt * LT : (lt + 1) * LT], ps[:],
                mb[0:1, lt * LT : (lt + 1) * LT],
            )

        # softmax pieces on partition 0
        negmx = small.tile([1, 1], F32, tag="negmx")
        nc.vector.tensor_reduce(negmx[:], scores[:], axis=AX.X, op=OP.max,
                                negate=True)
        e = small.tile([1, L], F32, tag="scores")
        ssum = small.tile([1, 1], F32, tag="ssum")
        nc.scalar.activation(
            e[:], scores[:], AF.Exp, bias=negmx[0:1, 0:1], scale=1.0,
            accum_out=ssum[0:1, 0:1],
        )
        rs = small.tile([1, 1], F32, tag="rs")
        nc.vector.reciprocal(rs[:], ssum[:])

        # transpose e (unnormalized weights) into the partition dim
        pwT = p_wt.tile([128, NLC], F32)
        for i in range(NLC):
            nc.tensor.matmul(
                pwT[:, i : i + 1], e[0:1, i * 128 : (i + 1) * 128],
                one[0:1, 0:1], is_transpose=True, start=True, stop=True,
            )
        ewT = small.tile([128, NLC], F32R, tag="ewT")
        nc.scalar.copy(ewT[:], pwT[:])

        # pass 2: ctx = (e @ keys) * (1/sum e)
        pctx0 = p_ctx.tile([1, LT], F32, tag="pctx")
        pctx1 = p_ctx.tile([1, LT], F32, tag="pctx")
        for lc in range(NLC):
            kN = knpool.tile([128, H], F32R)
            nc.sync.dma_start(
                kN[:], d["keysN"][b, lc * 128 : (lc + 1) * 128, :].bitcast(F32R)
            )
            nc.tensor.matmul(
                pctx0[:], ewT[:, lc : lc + 1], kN[:, 0:LT],
                start=(lc == 0), stop=(lc == NLC - 1),
            )
            nc.tensor.matmul(
                pctx1[:], ewT[:, lc : lc + 1], kN[:, LT : 2 * LT],
                start=(lc == 0), stop=(lc == NLC - 1),
            )
        cx = small.tile([1, H], F32, tag="cx")
        nc.scalar.activation(cx[0:1, 0:LT], pctx0[:], AF.Copy, bias=0.0,
                             scale=rs[0:1, 0:1])
        nc.scalar.activation(cx[0:1, LT : 2 * LT], pctx1[:], AF.Copy, bias=0.0,
                             scale=rs[0:1, 0:1])
        nc.sync.dma_start(d["out"][b : b + 1, :], cx[:])


def build():
    nc = bacc.Bacc("TRN2", target_bir_lowering=False, debug=False,
                   num_devices=NCORES)
    d = {
        "keysT": nc.dram_tensor("keysT", [BC, H, L], F32, kind="ExternalInput").ap(),
        "keysN": nc.dram_tensor("keysN", [BC, L, H], F32, kind="ExternalInput").ap(),
        "uawT": nc.dram_tensor("uawT", [128, HC * H], F32, kind="ExternalInput").ap(),
        "vaT": nc.dram_tensor("vaT", [128, HC], F32, kind="ExternalInput").ap(),
        "biasT": nc.dram_tensor("biasT", [128, HC * BC], F32, kind="ExternalInput").ap(),
        "one": nc.dram_tensor("one", [1, 1], F32, kind="ExternalInput").ap(),
        "maskb": nc.dram_tensor("maskb", [BC, L], F32, kind="ExternalInput").ap(),
        "out": nc.dram_tensor("out", [BC, H], F32, kind="ExternalOutput").ap(),
    }
    with tile.TileContext(nc) as tc, ExitStack() as ctx:
        _body(nc, tc, ctx, d)
    nc.compile()
    return nc


def _maybe_install_profile_hook():
    """BASS_TRACE=1 profiling under axon needs antenv.axon_hooks, which this
    image lacks; shim it with an in-memory module wired to libaxon_pjrt."""
    import sys, types
    if "antenv.axon_hooks" in sys.modules:
        return
    mod = types.ModuleType("antenv.axon_hooks")
    holder = [None]
    mod.set_axon_ntff_profile_hook = lambda h: holder.__setitem__(0, h)
    mod.get_axon_ntff_profile_hook = lambda: holder[0]
    sys.modules["antenv.axon_hooks"] = mod
    try:
        from trn_agent_boot.trn_boot import _ntff_profile_via_ctypes
        mod.set_axon_ntff_profile_hook(
            _ntff_profile_via_ctypes("/opt/axon/libaxon_pjrt.so"))
    except Exception:
        pass


def make_in_maps(query, keys, mask, wa_w, wa_b, ua_w, ua_b, va_w, va_b):
    query = np.asarray(query, dtype=np.float32)
    keys = np.asarray(keys, dtype=np.float32)
    mask = np.asarray(mask)
    wa_w = np.asarray(wa_w, dtype=np.float32)
    wa_b = np.asarray(wa_b, dtype=np.float32)
    ua_b = np.asarray(ua_b, dtype=np.float32)
    ua_w = np.asarray(ua_w, dtype=np.float32)
    va_w = np.asarray(va_w, dtype=np.float32)

    # lhsT chunk layout: arr[p, hc*H + k] = W[k, hc*128 + p]
    uawT = np.ascontiguousarray(
        ua_w.T.reshape(HC, 128, H).transpose(1, 0, 2).reshape(128, HC * H))
    vaT = np.ascontiguousarray(va_w[0].reshape(HC, 128).T)
    one = np.ones((1, 1), dtype=np.float32)
    maskb = np.where(mask, np.float32(-1e30), np.float32(0.0)).astype(np.float32)
    keysT = np.ascontiguousarray(keys.transpose(0, 2, 1))  # [B, H, L]
    # wq + wa_b + ua_b on host (0.05% of the FLOPs)
    wq = query[:, 0, :] @ wa_w.T + wa_b + ua_b  # [B, H]

    in_maps = []
    for c in range(NCORES):
        bs = slice(c * BC, (c + 1) * BC)
        biasT = np.ascontiguousarray(
            wq[bs].T.reshape(HC, 128, BC).transpose(1, 0, 2).reshape(128, HC * BC))
        in_maps.append({
            "keysT": keysT[bs],
            "keysN": np.ascontiguousarray(keys[bs]),
            "uawT": uawT,
            "vaT": vaT,
            "biasT": biasT,
            "one": one,
            "maskb": np.ascontiguousarray(maskb[bs]),
        })
    return in_maps


def kernel(query, keys, mask, wa_w, wa_b, ua_w, ua_b, va_w, va_b):
    global _nc, LAST_RESULT
    if os.environ.get("BASS_TRACE"):
        _maybe_install_profile_hook()
    if _nc is None:
        _nc = build()
    in_maps = make_in_maps(query, keys, mask, wa_w, wa_b, ua_w, ua_b, va_w, va_b)
    res = run_bass_kernel_spmd(_nc, in_maps, list(range(NCORES)))
    LAST_RESULT = res
    out = np.concatenate([res.results[c]["out"] for c in range(NCORES)], axis=0)
    return np.ascontiguousarray(out[:, None, :].astype(np.float32))


# revision 5
# speedup vs baseline: 1.1588x; 1.1588x over previous
"""Bahdanau additive attention on 8 Trainium2 NeuronCores (Bass/Tile).

Reference computation (per batch b):
    wq   = query @ wa_w.T + wa_b                      # [1, H]
    uk   = keys  @ ua_w.T + ua_b                      # [L, H]
    s    = tanh(wq + uk) @ va_w.T + va_b              # [L]
    s    = where(mask, -inf, s)
    w    = softmax(s)                                 # [L]
    ctx  = w @ keys                                   # [1, H]

Sharding: data-parallel over batch B=32 -> 4 batches per core; small
weights replicated.  The heavy matmul runs on the PE in float32r
(full-rate fp32, tf32-like operand rounding).

Device-side structure (per core, BC=4 batches), v3 (online softmax):
  - keys arrive pre-transposed (host) as keysT [H, L]; the big matmul
    computes uk^T [k, l] so the per-batch wq[k]+wa_b[k]+ua_b[k] (tiny,
    host-precomputed) is a per-partition ACT bias fused into the tanh.
  - scores = va . tanh(.) is a PE matmul with va as a [128,1] stationary.
  - softmax is ONLINE per 512-wide l-tile (flash style): running max m,
    running sum ssum, running ctx^T accumulator acc[128(h%128), 8(hc)].
    The weighted key sum uses DVE tensor_tensor_reduce on the SAME keysT
    tiles pass 1 just consumed (keys are read from HBM exactly once);
    exp weights are partition-broadcast by GpSimd.  softmax shift
    invariance drops va_b; the mask enters as an additive bias on scores.
  - final: ctx^T * (1/ssum), PE-transpose [128,8]->[8,128], DMA out.
"""

import os
import numpy as np
from contextlib import ExitStack

import concourse.bass as bass  # noqa: F401
import concourse.bacc as bacc
import concourse.tile as tile
from concourse import mybir
from concourse.bass_utils import run_bass_kernel_spmd

B, L, H = 32, 2048, 1024
NCORES = 8
BC = B // NCORES          # batches per core
HC = H // 128             # 128-chunks of the hidden dim
LT = 512                  # l-tile width
NLT = L // LT

F32 = mybir.dt.float32
F32R = mybir.dt.float32r
AF = mybir.ActivationFunctionType
AX = mybir.AxisListType
OP = mybir.AluOpType

_nc = None
LAST_RESULT = None


def _body(nc, tc, ctx, d):
    consts = ctx.enter_context(tc.tile_pool(name="consts", bufs=1))
    kpool = ctx.enter_context(tc.tile_pool(name="kT", bufs=4))
    tpool = ctx.enter_context(tc.tile_pool(name="tk", bufs=3))
    small = ctx.enter_context(tc.tile_pool(name="small", bufs=2))
    p_uk = ctx.enter_context(tc.tile_pool(name="p_uk", bufs=3, space="PSUM"))
    p_sc = ctx.enter_context(tc.tile_pool(name="p_sc", bufs=2, space="PSUM"))
    p_tr = ctx.enter_context(tc.tile_pool(name="p_tr", bufs=2, space="PSUM"))

    # ---- constants / weights (uawT split per-hc so compute starts early) ----
    uawT = consts.tile([128, HC, H], F32R)
    for hc in range(HC):
        nc.sync.dma_start(uawT[:, hc, :],
                          d["uawT"][:, hc * H : (hc + 1) * H].bitcast(F32R))
    vaT = consts.tile([128, HC], F32R)
    nc.sync.dma_start(vaT[:], d["vaT"].bitcast(F32R))
    biasT = consts.tile([128, HC * BC], F32)
    nc.sync.dma_start(biasT[:], d["biasT"])
    ident = consts.tile([128, 128], F32)
    nc.sync.dma_start(ident[:], d["ident"])

    for b in range(BC):
        mb = small.tile([1, L], F32, tag="mb")
        nc.sync.dma_start(mb[:], d["maskb"][b : b + 1, :])

        m = None      # running max           [1, 1]
        ssum = None   # running sum of exp    [1, 1]
        acc = None    # running ctx^T partials [128, HC]

        for lt in range(NLT):
            # ---- pass 1: uk^T -> tanh -> scores (PE + ACT) ----
            kT = kpool.tile([128, HC, LT], F32R)
            for hc in range(HC):
                nc.sync.dma_start(
                    kT[:, hc, :],
                    d["keysT"][b, hc * 128 : (hc + 1) * 128,
                               lt * LT : (lt + 1) * LT].bitcast(F32R),
                )
            ps = p_sc.tile([1, LT], F32)
            for kc in range(HC):
                pu = p_uk.tile([128, LT], F32)
                for hc in range(HC):
                    nc.tensor.matmul(
                        pu[:],
                        uawT[:, hc, kc * 128 : (kc + 1) * 128],
                        kT[:, hc, :],
                        start=(hc == 0),
                        stop=(hc == HC - 1),
                    )
                tk = tpool.tile([128, LT], F32R)
                nc.scalar.activation(
                    tk[:], pu[:], AF.Tanh,
                    bias=biasT[:, kc * BC + b : kc * BC + b + 1], scale=1.0,
                )
                nc.tensor.matmul(
                    ps[:], vaT[:, kc : kc + 1], tk[:],
                    start=(kc == 0), stop=(kc == HC - 1),
                )

            # ---- online softmax update (DVE/ACT/POOL) ----
            sm = small.tile([1, LT], F32, tag="sm")
            nc.vector.tensor_add(sm[:], ps[:], mb[0:1, lt * LT : (lt + 1) * LT])
            mx = small.tile([1, 1], F32, tag="mx")
            nc.vector.tensor_reduce(mx[:], sm[:], axis=AX.X, op=OP.max)
            if lt == 0:
                m_new = mx
            else:
                m_new = small.tile([1, 1], F32, tag="m")
                nc.vector.tensor_tensor(m_new[:], m[:], mx[:], op=OP.max)
            negm = small.tile([1, 1], F32, tag="negm")
            nc.vector.tensor_scalar_mul(negm[:], m_new[:], -1.0)

            e = small.tile([1, LT], F32, tag="e")
            s_lt = small.tile([1, 1], F32, tag="s_lt")
            nc.scalar.activation(e[:], sm[:], AF.Exp, bias=negm[0:1, 0:1],
                                 scale=1.0, accum_out=s_lt[0:1, 0:1])
            wb = small.tile([128, LT], F32, tag="wb")
            nc.gpsimd.partition_broadcast(wb[:], e[0:1, :])

            pp = small.tile([128, HC], F32, tag="pp")
            dst = pp if lt > 0 else None
            if lt == 0:
                acc_new = small.tile([128, HC], F32, tag="acc")
                dst = acc_new
            for hc in range(HC):
                dump = small.tile([128, LT], F32, tag="dump")
                nc.vector.scalar_tensor_tensor(
                    dump[:],
                    kT[:, hc, :].bitcast(F32),
                    1.0,
                    wb[:],
                    op0=OP.mult,
                    op1=OP.mult,
                    accum_out=dst[:, hc : hc + 1],
                )

            if lt == 0:
                ssum_new = small.tile([1, 1], F32, tag="ssum")
                nc.vector.tensor_copy(ssum_new[:], s_lt[:])
            else:
                f = small.tile([1, 1], F32, tag="f")
                nc.scalar.activation(f[:], m[:], AF.Exp, bias=negm[0:1, 0:1],
                                     scale=1.0)
                ssum_new = small.tile([1, 1], F32, tag="ssum")
                nc.vector.scalar_tensor_tensor(
                    ssum_new[:], ssum[:], f[0:1, 0:1], s_lt[:],
                    op0=OP.mult, op1=OP.add,
                )
                f_b = small.tile([128, 1], F32, tag="f_b")
                nc.gpsimd.partition_broadcast(f_b[:], f[0:1, :])
                acc_new = small.tile([128, HC], F32, tag="acc")
                nc.vector.scalar_tensor_tensor(
                    acc_new[:], acc[:], f_b[:, 0:1], pp[:],
                    op0=OP.mult, op1=OP.add,
                )
            m, ssum, acc = m_new, ssum_new, acc_new

        # ---- finalize: ctx = acc^T / ssum ----
        rs = small.tile([1, 1], F32, tag="rs")
        nc.vector.reciprocal(rs[:], ssum[:])
        rs_b = small.tile([128, 1], F32, tag="rs_b")
        nc.gpsimd.partition_broadcast(rs_b[:], rs[0:1, :])
        ctxT = small.tile([128, HC], F32, tag="ctxT")
        nc.vector.tensor_scalar_mul(ctxT[:], acc[:], rs_b[:, 0:1])
        tr = p_tr.tile([HC, 128], F32)
        nc.tensor.transpose(tr[:], ctxT[:], ident[:])
        cxrow = small.tile([HC, 128], F32, tag="cxrow")
        nc.scalar.copy(cxrow[:], tr[:])
        nc.sync.dma_start(
            d["out"][b : b + 1, :].rearrange("o (hc k) -> (o hc) k", k=128),
            cxrow[:],
        )


def build():
    nc = bacc.Bacc("TRN2", target_bir_lowering=False, debug=False,
                   num_devices=NCORES)
    d = {
        "keysT": nc.dram_tensor("keysT", [BC, H, L], F32, kind="ExternalInput").ap(),
        "uawT": nc.dram_tensor("uawT", [128, HC * H], F32, kind="ExternalInput").ap(),
        "vaT": nc.dram_tensor("vaT", [128, HC], F32, kind="ExternalInput").ap(),
        "biasT": nc.dram_tensor("biasT", [128, HC * BC], F32, kind="ExternalInput").ap(),
        "ident": nc.dram_tensor("ident", [128, 128], F32, kind="ExternalInput").ap(),
        "maskb": nc.dram_tensor("maskb", [BC, L], F32, kind="ExternalInput").ap(),
        "out": nc.dram_tensor("out", [BC, H], F32, kind="ExternalOutput").ap(),
    }
    with tile.TileContext(nc) as tc, ExitStack() as ctx:
        _body(nc, tc, ctx, d)
    nc.compile()
    return nc


def _maybe_install_profile_hook():
    """BASS_TRACE=1 profiling under axon needs antenv.axon_hooks, which this
    image lacks; shim it with an in-memory module wired to libaxon_pjrt."""
    import sys, types
    if "antenv.axon_hooks" in sys.modules:
        return
    mod = types.ModuleType("antenv.axon_hooks")
    holder = [None]
    mod.set_axon_ntff_profile_hook = lambda h: holder.__setitem__(0, h)
    mod.get_axon_ntff_profile_hook = lambda: holder[0]
    sys.modules["antenv.axon_hooks"] = mod
    try:
        from trn_agent_boot.trn_boot import _ntff_profile_via_ctypes
        mod.set_axon_ntff_profile_hook(
            _ntff_profile_via_ctypes("/opt/axon/libaxon_pjrt.so"))
    except Exception:
        pass


def make_in_maps(query, keys, mask, wa_w, wa_b, ua_w, ua_b, va_w, va_b):
    query = np.asarray(query, dtype=np.float32)
    keys = np.asarray(keys, dtype=np.float32)
    mask = np.asarray(mask)
    wa_w = np.asarray(wa_w, dtype=np.float32)
    wa_b = np.asarray(wa_b, dtype=np.float32)
    ua_b = np.asarray(ua_b, dtype=np.float32)
    ua_w = np.asarray(ua_w, dtype=np.float32)
    va_w = np.asarray(va_w, dtype=np.float32)

    # lhsT chunk layout: arr[p, hc*H + k] = W[k, hc*128 + p]
    uawT = np.ascontiguousarray(
        ua_w.T.reshape(HC, 128, H).transpose(1, 0, 2).reshape(128, HC * H))
    vaT = np.ascontiguousarray(va_w[0].reshape(HC, 128).T)
    ident = np.eye(128, dtype=np.float32)
    maskb = np.where(mask, np.float32(-1e30), np.float32(0.0)).astype(np.float32)
    keysT = np.ascontiguousarray(keys.transpose(0, 2, 1))  # [B, H, L]
    # wq + wa_b + ua_b on host (0.05% of the FLOPs)
    wq = query[:, 0, :] @ wa_w.T + wa_b + ua_b  # [B, H]

    in_maps = []
    for c in range(NCORES):
        bs = slice(c * BC, (c + 1) * BC)
        biasT = np.ascontiguousarray(
            wq[bs].T.reshape(HC, 128, BC).transpose(1, 0, 2).reshape(128, HC * BC))
        in_maps.append({
            "keysT": keysT[bs],
            "uawT": uawT,
            "vaT": vaT,
            "biasT": biasT,
            "ident": ident,
            "maskb": np.ascontiguousarray(maskb[bs]),
        })
    return in_maps


def kernel(query, keys, mask, wa_w, wa_b, ua_w, ua_b, va_w, va_b):
    global _nc, LAST_RESULT
    if os.environ.get("BASS_TRACE"):
        _maybe_install_profile_hook()
    if _nc is None:
        _nc = build()
    in_maps = make_in_maps(query, keys, mask, wa_w, wa_b, ua_w, ua_b, va_w, va_b)
    res = run_bass_kernel_spmd(_nc, in_maps, list(range(NCORES)))
    LAST_RESULT = res
    out = np.concatenate([res.results[c]["out"] for c in range(NCORES)], axis=0)
    return np.ascontiguousarray(out[:, None, :].astype(np.float32))
